# revision 2
# baseline (speedup 1.0000x reference)
"""Multi-head causal attention (B=4, T=2048, C=1024, H=16, DH=64) on 8 trn2 cores. v2.

Sharding: core c owns batch b = c//2 and heads [8*(c%2), 8*(c%2)+8)  (DP over B x TP over H).

Per-core device kernel:
  proj q/k: head-pair packed stationary [K=128, M=128] (two heads' DH side by side)
            -> psum [128,512] pair-stacked q^T/k^T; Pool copy -> bf16 SBUF.
  proj v:   natural [t, (h d)] psum; Pool copy+cast -> fp8 vsb8 [128, tt2, kt, h, 65]
            (col 64 = ones for rowsum).
  attention per (head-pair j, q-chunk c of 512): causal k-tiles r < 4c+4:
    S^T [128, head, 512] via two tile_position matmuls (bf16, K=64), N trimmed to
    valid causal window; ACT exp (scale 1/8) -> fp8 P8 [128, kt, head, 512] with
    trimmed windows; Pool memsets zero the invalid k-tile slivers of diag pairs;
    DVE tril mask on diag blocks; PV = fp8 DoubleRow matmul over k-tile pairs:
    po[65, 512] += vsb8[:, p, :, h, :].T (.) P8[:, :, h, :]  (row 64 = rowsum).
  phase 3 per (head, chunk): Pool copy po->SBUF, PE transpose to natural [t, d],
    DVE reciprocal of rowsums + broadcast multiply, DMA out fp32.
Projections of pair j+1 are interleaved into pair j's attention stream to keep
PE busy while ACT drains the exp backlog.
"""

import os
import numpy as np
import ml_dtypes

B, T, C, H, DH = 4, 2048, 1024, 16, 64
NCORES = 8
HPC = H // 2      # 8 heads per core
NP = HPC // 2     # 4 head pairs per core
CT = C // 128     # 8 contraction tiles
TC = T // 512     # 4 q-chunks
TK = T // 128     # 16 k-tiles

_cache = {}


def build_program(loop_n=1):
    import concourse.bass as bass
    import concourse.bacc as bacc
    import concourse.mybir as mybir
    import concourse.tile as tile
    from concourse.masks import make_upper_triangular, make_identity
    from contextlib import ExitStack

    f32 = mybir.dt.float32
    bf16 = mybir.dt.bfloat16
    f8 = mybir.dt.float8e4
    EXP = mybir.ActivationFunctionType.Exp
    DR = mybir.MatmulPerfMode.DoubleRow

    nc = bacc.Bacc("TRN2", target_bir_lowering=False, debug=False, num_devices=NCORES)
    xT_d = nc.dram_tensor("xT", [128, CT, T], bf16, kind="ExternalInput")
    x8_d = nc.dram_tensor("x8", [128, CT // 2, 2, T], f8, kind="ExternalInput")
    wq8_d = nc.dram_tensor("wq8", [128, CT // 2, 2, NP, 128], f8, kind="ExternalInput")
    wk8_d = nc.dram_tensor("wk8", [128, CT // 2, 2, NP, 128], f8, kind="ExternalInput")
    wq_d = nc.dram_tensor("wq", [128, CT, NP, 128], bf16, kind="ExternalInput")
    wk_d = nc.dram_tensor("wk", [128, CT, NP, 128], bf16, kind="ExternalInput")
    wv_d = nc.dram_tensor("wv", [128, CT, HPC, DH], bf16, kind="ExternalInput")
    out_d = nc.dram_tensor("out", [HPC, T, DH], f32, kind="ExternalOutput")

    with tile.TileContext(nc) as tc, ExitStack() as ctx:
        persist = ctx.enter_context(tc.tile_pool(name="persist", bufs=1))
        ptp = ctx.enter_context(tc.tile_pool(name="ptp", bufs=3))
        normp = ctx.enter_context(tc.tile_pool(name="normp", bufs=3))
        ptp16 = ctx.enter_context(tc.tile_pool(name="ptp16", bufs=2))
        pp = ctx.enter_context(tc.tile_pool(name="pp", bufs=2, space="PSUM"))
        pssp = ctx.enter_context(tc.tile_pool(name="pssp", bufs=2, space="PSUM"))
        pop = ctx.enter_context(tc.tile_pool(name="pop", bufs=2, space="PSUM"))

        # persistent SBUF
        xT = persist.tile([128, CT, T], bf16, tag="xT")
        x8 = persist.tile([128, CT // 2, 2, T], f8, tag="x8")
        wq8 = persist.tile([128, CT // 2, 2, NP, 128], f8, tag="wq8")
        wk8 = persist.tile([128, CT // 2, 2, NP, 128], f8, tag="wk8")
        wq = persist.tile([128, CT, NP, 128], bf16, tag="wq")
        wk = persist.tile([128, CT, NP, 128], bf16, tag="wk")
        wv = persist.tile([128, CT, HPC, DH], bf16, tag="wv")
        qT = persist.tile([128, NP, T], bf16, tag="qT")
        kT = persist.tile([128, NP, T], bf16, tag="kT")
        # v in fp8: [key-tile pair, kt-in-pair, head, DH+1]; col DH = 1.0 (rowsum)
        vsb8 = persist.tile([128, TK // 2, 2, HPC, DH + 2], f8, tag="vsb8")
        vsb16 = persist.tile([128, 4, HPC, DH + 2], bf16, tag="vsb16")
        trilT = persist.tile([128, 128], bf16, tag="trilT")
        tril2 = persist.tile([128, 2, 128], f8, tag="tril2")
        tril2b = persist.tile([128, 2, 128], bf16, tag="tril2b")
        ident = persist.tile([128, 128], f32, tag="ident")
        nbias = persist.tile([128, 1], f32, tag="nbias")

        # constants (outside the timing loop)
        make_upper_triangular(nc, trilT[:, :], val=1.0, diag=True)
        make_identity(nc, ident[:, :])
        nc.vector.tensor_copy(tril2[:, 0, :], trilT[:, :])
        nc.vector.tensor_copy(tril2[:, 1, :], trilT[:, :])
        nc.vector.tensor_copy(tril2b[:, 0, :], trilT[:, :])
        nc.vector.tensor_copy(tril2b[:, 1, :], trilT[:, :])
        nc.gpsimd.memset(vsb8[:, :, :, :, :], 1.0)
        nc.gpsimd.memset(vsb16[:, :, :, :], 1.0)
        nc.gpsimd.memset(nbias[:, :], -2.0)

        def emit_proj_qk_step(j, c, w, dst, ct, box):
            """One accumulation step of a q/k projection group (pair j, chunk c).
            Chunk 0 runs in bf16 (exact path); chunks 1+ use fp8 DoubleRow
            over contraction-tile pairs. All weights carry a 32x scale that
            the exp scale divides back out."""
            if ct == 0:
                box["ps"] = pp.tile([128, 512], f32, name="ps", tag="pp")
            ps = box["ps"]
            if c == 0:
                nc.tensor.matmul(
                    ps[:, :], w[0][:, ct, j, :], xT[:, ct, bass.ts(c, 512)],
                    start=(ct == 0), stop=(ct == CT - 1), skip_group_check=True,
                )
                last = ct == CT - 1
            else:
                nc.tensor.matmul(
                    ps[:, :], w[1][:, ct, :, j, :], x8[:, ct, :, bass.ts(c, 512)],
                    start=(ct == 0), stop=(ct == CT // 2 - 1),
                    perf_mode=DR, skip_group_check=True,
                )
                last = ct == CT // 2 - 1
            if last:
                nc.vector.tensor_copy(dst[:, j, bass.ts(c, 512)], ps[:, :])

        def emit_proj_v_step(tt, ct, box):
            if ct == 0:
                box["ps"] = pp.tile([128, 512], f32, name="ps", tag="pp")
            ps = box["ps"]
            nc.tensor.matmul(
                ps[:, :], xT[:, ct, bass.ts(tt, 128)], wv[:, ct, :, :],
                start=(ct == 0), stop=(ct == CT - 1), skip_group_check=True,
            )
            if ct == CT - 1:
                nc.vector.tensor_copy(
                    vsb8[:, tt // 2, tt % 2, :, 0:DH],
                    ps[:, :].rearrange("p (h d) -> p h d", h=HPC),
                )
                if tt < 4:
                    # bf16 copy of the first 4 key tiles for the exact PV
                    # path of q-chunk 0
                    nc.vector.tensor_copy(
                        vsb16[:, tt, :, 0:DH],
                        ps[:, :].rearrange("p (h d) -> p h d", h=HPC),
                    )

        # PE filler queue: projection work interleaved into the attention
        # stream so PE stays busy while ACT drains the exp backlog. Each
        # entry is (deadline_chunk_idx, closure).
        fillers = []

        def pull_fillers(n):
            k = 0
            while fillers and k < n:
                fillers.pop(0)[1]()
                k += 1

        def drain_fillers(idx):
            while fillers and fillers[0][0] <= idx:
                fillers.pop(0)[1]()

        def qk_closures(j):
            out = []
            for c in range(TC):
                for w, dst in (((wq, wq8), qT), ((wk, wk8), kT)):
                    box = {}
                    for ct in range(CT if c == 0 else CT // 2):
                        out.append(
                            lambda j=j, c=c, w=w, dst=dst, ct=ct, box=box:
                            emit_proj_qk_step(j, c, w, dst, ct, box))
            return out

        def v_closures(tts):
            out = []
            for tt in tts:
                box = {}
                for ct in range(CT):
                    out.append(
                        lambda tt=tt, ct=ct, box=box:
                        emit_proj_v_step(tt, ct, box))
            return out

        def emit_attention_chunk(j, c):
            hA, hB = 2 * j, 2 * j + 1
            nr = 4 * c + 4
            fp8path = c > 0
            poA = pop.tile([DH + 2, 512], f32, tag="po")
            poB = pop.tile([DH + 2, 512], f32, tag="po")
            pss = {}
            p8s = {}

            def emit_S(r):
                vlo = max(0, r - 4 * c) * 128
                ps = pssp.tile([128, 2, 512], f32, tag="pss")
                pss[r] = ps
                nc.tensor.matmul(
                    ps[:, 0, vlo:512], kT[0:64, j, bass.ts(r, 128)],
                    qT[0:64, j, 512 * c + vlo:512 * (c + 1)],
                    start=True, stop=True, tile_position=(0, 0),
                )
                nc.tensor.matmul(
                    ps[:, 1, vlo:512], kT[64:128, j, bass.ts(r, 128)],
                    qT[64:128, j, 512 * c + vlo:512 * (c + 1)],
                    start=True, stop=True, tile_position=(64, 0),
                )

            def emit_EXP(p):
                """exp for k-tile pair p -> P8 [128, kt, head, q] (fp8, or
                bf16 for the exact chunk-0 path)."""
                r0, r1 = 2 * p, 2 * p + 1
                if fp8path:
                    p8 = ptp.tile([128, 2, 2, 512], f8, tag="p8")
                else:
                    p8 = ptp16.tile([128, 2, 2, 512], bf16, tag="p16")
                p8s[p] = p8
                for i, r in ((0, r0), (1, r1)):
                    vlo = max(0, r - 4 * c) * 128
                    # bias -2 keeps exp within fp8e4m3 range; it cancels in
                    # the rowsum normalization
                    nc.scalar.activation(
                        p8[:, i, :, vlo:512], pss[r][:, :, vlo:512],
                        EXP, scale=0.125 / 1024.0, bias=nbias[:, 0:1])
                # zero the sliver where k-tile r1 is wholly masked but inside
                # the pair's PV window (fp8 DoubleRow shares one window for
                # both k-tiles), and tril-mask the diagonal blocks
                jd0, jd1 = r0 - 4 * c, r1 - 4 * c
                if fp8path and jd1 >= 0:
                    vlo0 = max(0, jd0) * 128
                    nc.vector.memset(p8[:, 1, :, vlo0:vlo0 + 128], 0.0)
                for i, jd in ((0, jd0), (1, jd1)):
                    if jd >= 0:
                        tm = tril2 if fp8path else tril2b
                        nc.vector.tensor_mul(
                            p8[:, i, :, bass.ts(jd, 128)],
                            p8[:, i, :, bass.ts(jd, 128)],
                            tm[:, :, :])
                pss.pop(r0)
                pss.pop(r1)

            def emit_PV(p):
                p8 = p8s.pop(p)
                if fp8path:
                    vlo = max(0, 2 * p - 4 * c) * 128
                    for po, h in ((poA, 0), (poB, 1)):
                        nc.tensor.matmul(
                            po[:, vlo:512], vsb8[:, p, :, 2 * j + h, :],
                            p8[:, :, h, vlo:512],
                            start=(p == 0), stop=(p == nr // 2 - 1),
                            perf_mode=DR, skip_group_check=True,
                        )
                else:
                    for i, r in ((0, 2 * p), (1, 2 * p + 1)):
                        vlo = max(0, r - 4 * c) * 128
                        for po, h in ((poA, 0), (poB, 1)):
                            nc.tensor.matmul(
                                po[:, vlo:512], vsb16[:, r, 2 * j + h, :],
                                p8[:, i, h, vlo:512],
                                start=(r == 0), stop=(r == nr - 1),
                                skip_group_check=True,
                            )

            npair = nr // 2
            for p in range(npair):
                emit_S(2 * p)
                emit_S(2 * p + 1)
                emit_EXP(p)
                pull_fillers(4)
                if p >= 1:
                    emit_PV(p - 1)
            emit_PV(npair - 1)

            # normalize + write natural-layout output rows for this chunk
            for h, po in ((hA, poA), (hB, poB)):
                ou_s = normp.tile([DH + 1, 512], f32, tag="ou_s")
                nc.vector.tensor_copy(ou_s[:, :], po[0:DH + 1, :])
                pt = pop.tile([128, 4, DH + 1], f32, tag="po")
                for t4 in range(4):
                    nc.tensor.transpose(
                        pt[:, t4, 0:DH],
                        ou_s[0:DH, bass.ts(t4, 128)], ident[0:DH, 0:DH])
                    nc.tensor.transpose(
                        pt[:, t4, DH:DH + 1],
                        ou_s[DH:DH + 1, bass.ts(t4, 128)],
                        ident[DH:DH + 1, DH:DH + 1])
                rsc = normp.tile([128, 4], f32, tag="rsc")
                nc.vector.reciprocal(rsc[:, :], pt[:, :, DH:DH + 1].rearrange("p a b -> p (a b)"))
                on_t = normp.tile([128, 4, DH], f32, tag="on_t")
                for t4 in range(4):
                    nc.vector.tensor_scalar_mul(
                        on_t[:, t4, :], pt[:, t4, 0:DH], rsc[:, t4:t4 + 1])
                nc.sync.dma_start(
                    out_d[h, bass.ts(c, 512), :].rearrange(
                        "(t p) d -> p t d", p=128),
                    on_t[:, :, :])

        def body():
            # input DMAs, split per contraction tile so the first
            # projection matmuls can start as soon as ct=0 lands
            for ct in range(CT):
                nc.sync.dma_start(wq[:, ct, :, :], wq_d[:, ct, :, :])
                nc.sync.dma_start(wk[:, ct, :, :], wk_d[:, ct, :, :])
                nc.sync.dma_start(xT[:, ct, :], xT_d[:, ct, :])
            nc.sync.dma_start(wv[:, :, :, :], wv_d[:, :, :, :])
            nc.sync.dma_start(x8[:, :, :, :], x8_d[:, :, :, :])
            nc.sync.dma_start(wq8[:, :, :, :, :], wq8_d[:, :, :, :, :])
            nc.sync.dma_start(wk8[:, :, :, :, :], wk8_d[:, :, :, :, :])

            # pair-0 q/k and the first 4 v tiles up front
            fillers.clear()
            for f in qk_closures(0):
                f()
            for f in v_closures(range(4)):
                f()
            # filler queue, in deadline order: v tile tt must be emitted
            # before chunk (0, tt//4); pair j's q/k before chunk (j, 0)
            for tt in range(4, TK):
                for f in v_closures([tt]):
                    fillers.append((tt // 4, f))
            for jj in range(1, NP):
                for f in qk_closures(jj):
                    fillers.append((4 * jj, f))

            idx = 0
            for j in range(NP):
                for c in range(TC):
                    drain_fillers(idx)
                    emit_attention_chunk(j, c)
                    idx += 1
            drain_fillers(10 ** 9)

        nbody = int(os.environ.get("NBODY", "1"))
        if loop_n > 1:
            with tc.For_i(0, loop_n, 1):
                for _ in range(nbody):
                    body()
        else:
            body()

    nc.compile()
    return nc


def _prep_core_inputs(x, Wq, Wk, Wv, core):
    bf = ml_dtypes.bfloat16
    b = core // 2
    hs = (core % 2) * HPC
    xT = np.ascontiguousarray(x[b].T).astype(bf)          # [C, T]
    xT = xT.reshape(CT, 128, T).transpose(1, 0, 2)        # [128, CT, T]
    f8 = ml_dtypes.float8_e4m3
    ws = []
    w8s = []
    for W in (Wq, Wk):
        Wg = W[hs:hs + HPC] * 32.0                        # [HPC, C, DH], 32x scale
        # pair-packed: [C, NP, 128] with hA cols 0:64, hB cols 64:128
        w2 = np.concatenate(
            [np.stack([Wg[2 * j], Wg[2 * j + 1]], axis=0) for j in range(NP)],
            axis=0).reshape(NP, 2, C, DH)                 # [NP, 2, C, DH]
        w2 = w2.transpose(2, 0, 1, 3).reshape(C, NP, 128)  # [C, NP, 128]
        w2 = w2.reshape(CT, 128, NP, 128)                  # [CT, 128, NP, 128]
        ws.append(np.ascontiguousarray(w2.transpose(1, 0, 2, 3).astype(bf)))
        w8 = w2.reshape(CT // 2, 2, 128, NP, 128).transpose(2, 0, 1, 3, 4)
        w8s.append(np.ascontiguousarray(w8.astype(f8)))
    x8 = x[b].astype(f8)                                  # [T, C]
    x8 = np.ascontiguousarray(x8.T).reshape(CT // 2, 2, 128, T).transpose(2, 0, 1, 3)
    x8 = np.ascontiguousarray(x8)
    wv = Wv[hs:hs + HPC].transpose(1, 0, 2).astype(bf)    # [C, HPC, DH]
    wv = wv.reshape(CT, 128, HPC, DH).transpose(1, 0, 2, 3)
    return {
        "xT": np.ascontiguousarray(xT),
        "wq": ws[0], "wk": ws[1],
        "x8": x8, "wq8": w8s[0], "wk8": w8s[1],
        "wv": np.ascontiguousarray(wv),
    }


def run_on_device(inputs, loop_n=1, trace=False):
    from concourse.bass_utils import run_bass_kernel_spmd

    key = loop_n
    if key not in _cache:
        _cache[key] = build_program(loop_n)
    nc = _cache[key]
    in_maps = [
        _prep_core_inputs(inputs["x"], inputs["Wq"], inputs["Wk"], inputs["Wv"], c)
        for c in range(NCORES)
    ]
    res = run_bass_kernel_spmd(nc, in_maps, list(range(NCORES)), trace=trace)
    return res


def kernel(x, Wq, Wk, Wv):
    res = run_on_device({"x": x, "Wq": Wq, "Wk": Wk, "Wv": Wv})
    out = np.empty((B, T, H * DH), np.float32)
    for core in range(NCORES):
        b = core // 2
        hs = (core % 2) * HPC
        o = res.results[core]["out"]  # [HPC, T, DH]
        out[b, :, hs * DH:(hs + HPC) * DH] = o.transpose(1, 0, 2).reshape(T, HPC * DH)
    return out
